# revision 7
# baseline (speedup 1.0000x reference)
"""Trainium2 Bass kernel for nn_CriticNetwork (GCN message passing + critic MLP).

Strategy (8 NeuronCores, SPMD, no collectives):
  - Only agg[agent_idx] rows are consumed downstream, so message passing is
    pruned to edges whose destination is an agent node (dead-code elimination).
  - GCN transform is algebraically moved after aggregation:
    A_hat @ (x W) == (A_hat @ x) W, so we aggregate 128-dim x rows.
  - Agents are sharded evenly: core c owns agents [c*2048, (c+1)*2048), sorted
    by indegree so fixed-K slot tiles are tight. The host materializes each
    core's (dinv[src]-prescaled, bf16) edge-source rows into a dense slot
    tensor E (pure byte movement; indices are host-known), which the device
    streams in with large sequential DMAs. The device does all arithmetic:
    identity-matmul transpose+accumulate into PSUM (giving agg^T
    feature-major), dinv[dst] scaling, and the critic head entirely
    feature-major with float32r matmuls. LayerNorm mean-centering is folded
    into W1/W2 host-side (exact); LN gains/biases are folded into
    weights/bias vectors (exact given beta1 == 0, g1 > 0, which the module's
    init guarantees). ba+beta2 rides as an extra ones-row in the action GEMM.
  - Host does only index bookkeeping, byte layout, and weight folding; all
    O(E*dim) and O(A*dim^2) arithmetic runs on device.
"""
import os
import sys

sys.path.insert(0, "/opt/trn_rl_repo")

import numpy as np
import ml_dtypes

import concourse.bass as bass
import concourse.tile as tile
import concourse.mybir as mybir
from concourse import bacc
from concourse.bass_utils import run_bass_kernel_spmd

# ---- problem constants (hardcoded per spec) ----
N_NODES = 50000
DIM = 128          # IN_DIM
HID = 256
F1 = 1024
F2 = 512
NACT = 64
N_EDGES = 800000
N_AGENTS = 16384
N_CORES = 8
PA = N_AGENTS // N_CORES      # 2048 agents per core
TILES = PA // 128             # 16 d-tiles per core
GROUPS = 4                    # head processed in 4 groups of 512 agents
DG = PA // GROUPS             # 512
EPS = 1e-5

F32 = mybir.dt.float32
F32R = mybir.dt.float32r
BF16 = mybir.dt.bfloat16
AF = mybir.ActivationFunctionType
OP = mybir.AluOpType

_KERNEL_CACHE = {}


def _preprocess(x, action, W_gcn, b_gcn, W1, b1, g1, beta1, W2, b2, g2, beta2,
                Wa, ba, Wq, bq, edge_index, agent_idx):
    f32 = np.float32
    x = np.asarray(x, f32); action = np.asarray(action, f32)
    edge_index = np.asarray(edge_index); agent_idx = np.asarray(agent_idx)
    W_gcn = np.asarray(W_gcn, f32); b_gcn = np.asarray(b_gcn, f32)
    W1 = np.asarray(W1, f32); b1 = np.asarray(b1, f32)
    g1 = np.asarray(g1, f32); beta1 = np.asarray(beta1, f32)
    W2 = np.asarray(W2, f32); b2 = np.asarray(b2, f32)
    g2 = np.asarray(g2, f32); beta2 = np.asarray(beta2, f32)
    Wa = np.asarray(Wa, f32); ba = np.asarray(ba, f32)
    Wq = np.asarray(Wq, f32); bq = np.asarray(bq, f32)

    assert np.all(beta1 == 0.0) and np.all(g1 > 0.0), \
        "kernel fast path requires beta1==0 and g1>0 (module init guarantees this)"

    N = N_NODES
    loops = np.arange(N, dtype=edge_index.dtype)
    src_all = np.concatenate([edge_index[0], loops])
    dst_all = np.concatenate([edge_index[1], loops])
    deg = np.bincount(dst_all, minlength=N).astype(np.int64)
    dinv = (1.0 / np.sqrt(np.maximum(deg, 1.0))).astype(f32)

    order = np.argsort(dst_all, kind="stable")
    src_sorted = src_all[order]
    starts = np.searchsorted(dst_all[order], np.arange(N + 1))

    # per-core agent partition + indegree sort
    perms, agents_p, indegs = [], [], []
    for c in range(N_CORES):
        ag = agent_idx[c * PA:(c + 1) * PA]
        ind = deg[ag]
        perm = np.argsort(ind, kind="stable")
        perms.append(perm)
        agents_p.append(ag[perm])
        indegs.append(ind[perm])

    # shared per-tile K (max over cores so the SPMD program is identical)
    K = np.zeros(TILES, np.int64)
    for c in range(N_CORES):
        K = np.maximum(K, indegs[c].reshape(TILES, 128).max(axis=1))
    K = np.maximum(K, 1).astype(int)
    koff = np.concatenate([[0], np.cumsum(K)])
    tot_k = int(koff[-1])

    # prescaled node features (bf16), plus a zero pad row for empty slots
    xs = np.zeros((N + 1, DIM), ml_dtypes.bfloat16)
    xs[:N] = (x * dinv[:, None]).astype(ml_dtypes.bfloat16)

    # per-core slot tensor E: [128 agents, tot_k * 128 feat] bf16.
    # E[p, (koff[t]+k)*128 + f] = xs[tbl_t[k, p], f]  (zero row for pads)
    E_list, dinvd_list, actT_list = [], [], []
    for c in range(N_CORES):
        ag = agents_p[c]; ind = indegs[c]
        tbl = np.full((tot_k, 128), N, np.int64)   # N -> zero pad row
        for t in range(TILES):
            for p in range(128):
                a = int(ag[t * 128 + p]); d = int(ind[t * 128 + p])
                s = starts[a]
                tbl[koff[t]:koff[t] + d, p] = src_sorted[s:s + d]
        # gather rows -> [tot_k, 128 agents, 128 feat] -> [128, tot_k*128]
        Ec = xs[tbl]                                  # [tot_k, 128, DIM] bf16
        Ec = np.ascontiguousarray(Ec.transpose(1, 0, 2).reshape(128, tot_k * DIM))
        E_list.append(Ec)
        dinvd_list.append(np.broadcast_to(
            dinv[ag].reshape(1, PA), (128, PA)).astype(f32))
        actp = action[c * PA:(c + 1) * PA][perms[c]].T      # [64, PA]
        actT_list.append(np.ascontiguousarray(
            np.concatenate([actp, np.ones((1, PA), f32)], axis=0)))

    # ---- weight folding (exact algebra) ----
    w1m = W1.mean(axis=1)                       # [HID]
    W1f = W1 - w1m[:, None]                     # zero col-mean
    b1c = b1 - b1.mean()
    W2g = g1[:, None] * W2
    w2gm = W2g.mean(axis=1)
    W2f = W2g - w2gm[:, None]
    b2c = b2 - b2.mean()
    bb = ba + beta2

    def ktile_pack(W, kt, fdim):   # [kt*128, fdim] -> [128, kt*fdim]
        return np.ascontiguousarray(
            W.reshape(kt, 128, fdim).transpose(1, 0, 2).reshape(128, kt * fdim))

    weights = {
        "wgcn": W_gcn,                                    # [128, 256]
        "w1": ktile_pack(W1f, 2, F1),                     # [128, 2048]
        "w2": ktile_pack(W2f, 8, F2),                     # [128, 4096]
        "wa": np.ascontiguousarray(
            np.concatenate([Wa, bb[None, :]], axis=0)),   # [65, 512]
        "wq": np.ascontiguousarray(Wq.reshape(4, 128).T), # [128, 4]
        "bgcn_col": np.ascontiguousarray(b_gcn.reshape(2, 128).T),
        "b1_col": np.ascontiguousarray(b1c.reshape(8, 128).T),
        "b2c_col": np.ascontiguousarray(b2c.reshape(4, 128).T),
        "g2_col": np.ascontiguousarray(g2.reshape(4, 128).T),
        "onesmat_in": np.ones((128, 128), f32),
        "ident_in": np.eye(128, dtype=f32),
    }
    meta = dict(K=tuple(int(k) for k in K),
                koff=tuple(int(o) for o in koff), tot_k=tot_k,
                bq=float(bq[0]))
    percore = dict(E=E_list, dinvd=dinvd_list, actT=actT_list)
    return weights, percore, perms, meta


def _build(meta):
    K = meta["K"]; koff = meta["koff"]
    tot_k = meta["tot_k"]; bq = meta["bq"]
    KMAX = max(K)

    nc = bacc.Bacc("TRN2", target_bir_lowering=False, debug=False,
                   num_devices=N_CORES, num_swdge_queues=4)
    dram = {}
    def din(name, shape, dt):
        dram[name] = nc.dram_tensor(name, shape, dt, kind="ExternalInput").ap()
        return dram[name]

    E_d = din("E", [128, tot_k * DIM], BF16)
    dinvd = din("dinvd", [128, PA], F32)
    actT_d = din("actT", [NACT + 1, PA], F32)
    wgcn_d = din("wgcn", [128, HID], F32)
    w1_d = din("w1", [128, 2 * F1], F32)
    w2_d = din("w2", [128, 8 * F2], F32)
    wa_d = din("wa", [NACT + 1, F2], F32)
    wq_d = din("wq", [128, 4], F32)
    bgcn_d = din("bgcn_col", [128, 2], F32)
    b1_d = din("b1_col", [128, 8], F32)
    b2c_d = din("b2c_col", [128, 4], F32)
    g2_d = din("g2_col", [128, 4], F32)
    onesmat_d = din("onesmat_in", [128, 128], F32)
    ident_d = din("ident_in", [128, 128], F32)
    OUT = nc.dram_tensor("q", [1, PA], F32, kind="ExternalOutput").ap()

    with tile.TileContext(nc) as tc:
        with tc.tile_pool(name="w", bufs=1) as wp, \
             tc.tile_pool(name="edges", bufs=3) as ep, \
             tc.tile_pool(name="zp", bufs=3) as zp, \
             tc.tile_pool(name="s1p", bufs=10) as s1p, \
             tc.tile_pool(name="sqp", bufs=2) as sqp, \
             tc.tile_pool(name="yap", bufs=2) as yap, \
             tc.tile_pool(name="uup", bufs=6) as uup, \
             tc.tile_pool(name="u2p", bufs=2) as u2p, \
             tc.tile_pool(name="tlp", bufs=3) as tlp, \
             tc.tile_pool(name="sap", bufs=5) as sap, \
             tc.tile_pool(name="vec", bufs=8) as vec, \
             tc.tile_pool(name="bcp", bufs=3) as bcp, \
             tc.tile_pool(name="ps", bufs=1, space="PSUM") as pp:

            # ---------- preload (ident first; agg starts as soon as E0 lands) ----------
            ident = wp.tile([128, 128], BF16); nc.gpsimd.dma_start(ident[:], ident_d[:])
            wgcn = wp.tile([128, HID], F32R); nc.gpsimd.dma_start(wgcn[:], wgcn_d[:])
            bgcn = wp.tile([128, 2], F32); nc.gpsimd.dma_start(bgcn[:], bgcn_d[:])
            dinvd_b = wp.tile([128, PA], F32); nc.gpsimd.dma_start(dinvd_b[:], dinvd[:])
            w1 = wp.tile([128, 2 * F1], F32R); nc.gpsimd.dma_start(w1[:], w1_d[:])
            b1c = wp.tile([128, 8], F32); nc.gpsimd.dma_start(b1c[:], b1_d[:])
            onesm = wp.tile([128, 128], F32R); nc.gpsimd.dma_start(onesm[:], onesmat_d[:])
            w2 = wp.tile([128, 8 * F2], F32R); nc.gpsimd.dma_start(w2[:], w2_d[:])
            b2c = wp.tile([128, 4], F32); nc.gpsimd.dma_start(b2c[:], b2c_d[:])
            g2c = wp.tile([128, 4], F32); nc.gpsimd.dma_start(g2c[:], g2_d[:])
            wa = wp.tile([NACT + 1, F2], F32R); nc.gpsimd.dma_start(wa[:], wa_d[:])
            actT = wp.tile([NACT + 1, PA], F32R); nc.gpsimd.dma_start(actT[:], actT_d[:])
            wq = wp.tile([128, 4], F32R); nc.gpsimd.dma_start(wq[:], wq_d[:])
            agg = wp.tile([128, PA], F32R)       # agg^T, feature-major
            qrow = wp.tile([1, PA], F32)

            def stage_agg(g):
                # aggregation for group g's 4 d-tiles
                for tl in range(4):
                    t = g * 4 + tl
                    kt = K[t]
                    e = ep.tile([128, KMAX * 128], BF16, tag="edges")
                    nc.sync.dma_start(e[:, :kt * 128],
                                      E_d[:, koff[t] * 128:koff[t + 1] * 128])
                    aps = pp.tile([128, 128], F32, tag="agg", bufs=2)
                    for k in range(kt):
                        nc.tensor.matmul(aps[:], e[:, k * 128:(k + 1) * 128],
                                         ident[:], start=(k == 0),
                                         stop=(k == kt - 1))
                    nc.vector.tensor_tensor(
                        agg[:, t * 128:(t + 1) * 128], aps[:],
                        dinvd_b[:, t * 128:(t + 1) * 128], OP.mult)

            stage_agg(0)
            for g in range(GROUPS):
                gs0 = g * DG
                # ---------- transform: z = relu(W_gcn^T aggT + b_gcn) ----------
                zt = []
                for h in range(2):
                    zps = pp.tile([128, DG], F32, tag="big", bufs=3)
                    nc.tensor.matmul(zps[:], wgcn[:, h * 128:(h + 1) * 128],
                                     agg[:, gs0:gs0 + DG], start=True, stop=True)
                    z = zp.tile([128, DG], F32R, tag="z")
                    nc.scalar.activation(z[:], zps[:], AF.Relu,
                                         bias=bgcn[:, h:h + 1])
                    zt.append(z)

                # ---------- L1 + LN1 (mean folded into W1f/b1c) ----------
                ps_sq1 = pp.tile([128, DG], F32, tag="stat", bufs=2)
                s1r = []
                for c in range(8):
                    lp = pp.tile([128, DG], F32, tag="big", bufs=3)
                    nc.tensor.matmul(lp[:], w1[:, c * 128:c * 128 + 128],
                                     zt[0][:], start=True, stop=False)
                    nc.tensor.matmul(lp[:], w1[:, F1 + c * 128:F1 + c * 128 + 128],
                                     zt[1][:], start=False, stop=True)
                    sq = sqp.tile([128, DG], F32R, tag="sq")
                    nc.scalar.activation(sq[:], lp[:], AF.Square,
                                         bias=b1c[:, c:c + 1])
                    nc.tensor.matmul(ps_sq1[:], onesm[:], sq[:],
                                     start=(c == 0), stop=(c == 7))
                    sr = s1p.tile([128, DG], F32R, tag="s1")
                    nc.scalar.activation(sr[:], lp[:], AF.Relu,
                                         bias=b1c[:, c:c + 1])
                    s1r.append(sr)

                var1 = vec.tile([128, DG], F32, tag="v")
                nc.vector.tensor_scalar(var1[:], ps_sq1[:], 1.0 / F1, EPS,
                                        OP.mult, OP.add)
                std1 = vec.tile([128, DG], F32, tag="v")
                nc.scalar.activation(std1[:], var1[:], AF.Sqrt)
                rstd1b = bcp.tile([128, DG], F32, tag="bc")
                nc.vector.reciprocal_approx_fast(rstd1b[:], std1[:])

                # ---------- L2 + LN2 stats (u = x2 - mu2 materialized) ----------
                ps_u2 = pp.tile([128, DG], F32, tag="stat", bufs=2)
                us = []
                for c2 in range(4):
                    lp = pp.tile([128, DG], F32, tag="big", bufs=3)
                    for k8 in range(8):
                        nc.tensor.matmul(
                            lp[:], w2[:, k8 * F2 + c2 * 128:k8 * F2 + c2 * 128 + 128],
                            s1r[k8][:], start=(k8 == 0), stop=(k8 == 7))
                    ya = yap.tile([128, DG], F32, tag="ya")
                    nc.vector.tensor_tensor(ya[:], lp[:], rstd1b[:], OP.mult)
                    u = uup.tile([128, DG], F32R, tag="u")
                    nc.scalar.activation(u[:], ya[:], AF.Identity,
                                         bias=b2c[:, c2:c2 + 1])
                    u2 = u2p.tile([128, DG], F32R, tag="u2")
                    nc.gpsimd.tensor_tensor(u2[:], u[:], u[:], OP.mult)
                    nc.tensor.matmul(ps_u2[:], onesm[:], u2[:],
                                     start=(c2 == 0), stop=(c2 == 3))
                    us.append(u)

                var2 = vec.tile([128, DG], F32, tag="v")
                nc.vector.tensor_scalar(var2[:], ps_u2[:], 1.0 / F2, EPS,
                                        OP.mult, OP.add)
                std2 = vec.tile([128, DG], F32, tag="v")
                nc.scalar.activation(std2[:], var2[:], AF.Sqrt)
                rstd2b = bcp.tile([128, DG], F32, tag="bc")
                nc.vector.reciprocal_approx_fast(rstd2b[:], std2[:])

                # issue next group's aggregation now: its id-matmuls keep the
                # tensor engine busy while this group's LN2 chain resolves
                if g + 1 < GROUPS:
                    stage_agg(g + 1)

                # ---------- tail: sa = relu(g2*u*rstd2 + (Wa a + ba + beta2)) ----------
                sas = []
                for c2 in range(4):
                    pa = pp.tile([128, DG], F32, tag="big", bufs=3)
                    nc.tensor.matmul(pa[:], wa[:, c2 * 128:(c2 + 1) * 128],
                                     actT[:, gs0:gs0 + DG], start=True, stop=True)
                    wv = tlp.tile([128, DG], F32, tag="tl")
                    nc.vector.tensor_tensor(wv[:], us[c2][:], rstd2b[:], OP.mult)
                    t2 = tlp.tile([128, DG], F32, tag="tl")
                    nc.vector.scalar_tensor_tensor(t2[:], wv[:], g2c[:, c2:c2 + 1],
                                                   pa[:], OP.mult, OP.add)
                    sa = sap.tile([128, DG], F32R, tag="sa")
                    nc.scalar.activation(sa[:], t2[:], AF.Relu)
                    sas.append(sa)

                # ---------- final: q = sa @ Wq + bq (M=1 orientation) ----------
                qp = pp.tile([1, DG], F32, tag="q", bufs=1)
                for c2 in range(4):
                    nc.tensor.matmul(qp[:], wq[:, c2:c2 + 1], sas[c2][:],
                                     start=(c2 == 0), stop=(c2 == 3))
                nc.scalar.activation(qrow[:, gs0:gs0 + DG], qp[:], AF.Copy,
                                     bias=bq)

            nc.sync.dma_start(OUT[:], qrow[:])
    nc.compile()
    return nc


def kernel(**inputs):
    weights, percore, perms, meta = _preprocess(**inputs)

    key = (meta["K"], meta["tot_k"])
    if key not in _KERNEL_CACHE:
        _KERNEL_CACHE[key] = _build(meta)
    nc = _KERNEL_CACHE[key]

    in_maps = []
    for c in range(N_CORES):
        m = dict(weights)
        m["E"] = percore["E"][c]
        m["dinvd"] = percore["dinvd"][c]
        m["actT"] = percore["actT"][c]
        in_maps.append(m)

    trace = os.environ.get("KERNEL_TRACE", "0") == "1"
    kw = {}
    if trace:
        import types, contextlib, ctypes
        if "antenv.axon_hooks" not in sys.modules:
            lib = ctypes.CDLL("/opt/axon/libaxon_pjrt.so")
            lib.axon_start_nrt_profile.argtypes = [
                ctypes.POINTER(ctypes.c_int64), ctypes.c_size_t]
            lib.axon_start_nrt_profile.restype = ctypes.c_int64
            lib.axon_stop_nrt_profile.argtypes = [ctypes.c_char_p]
            lib.axon_stop_nrt_profile.restype = ctypes.c_int64

            @contextlib.contextmanager
            def _hook(output_dir, device_ids):
                import jax
                jax.devices()
                if device_ids:
                    ids = (ctypes.c_int64 * len(device_ids))(*device_ids)
                    rc = lib.axon_start_nrt_profile(ids, len(device_ids))
                else:
                    rc = lib.axon_start_nrt_profile(None, 0)
                if rc != 0:
                    raise RuntimeError(f"axon_start_nrt_profile rc={rc}")
                try:
                    yield
                finally:
                    n = lib.axon_stop_nrt_profile(str(output_dir).encode())
                    print(f"profile: {n} file(s) written to {output_dir}",
                          file=sys.stderr)

            mod = types.ModuleType("antenv.axon_hooks")
            mod.get_axon_ntff_profile_hook = lambda: _hook
            sys.modules["antenv.axon_hooks"] = mod
        kw = dict(trace=True,
                  tmpdir=os.environ.get("KERNEL_TRACE_DIR") or None)

    res = run_bass_kernel_spmd(nc, in_maps, list(range(N_CORES)), **kw)
    if trace and res.exec_time_ns is not None:
        print(f"HW exec time: {res.exec_time_ns} ns")

    out = np.empty((N_AGENTS, 1), np.float32)
    for c in range(N_CORES):
        q = res.results[c]["q"].reshape(PA, 1)   # indegree-sorted order
        blk = out[c * PA:(c + 1) * PA]
        blk[perms[c]] = q
    return out
